# revision 15
# baseline (speedup 1.0000x reference)
"""DeepSet baseline kernel for Trainium2 (8 NeuronCores, data-parallel).

Model (reference):
    mask  = sign(|sum_e words|)                  # padding rows are all-zero
    h1    = tanh(words @ W1 + b1)                # [B,S,H]
    h2    = tanh(h1 @ W2 + b2)                   # [B,S,H]
    enc   = h2 @ W3 + b3                         # [B,S,C]
    codes = sum_s enc * mask                     # [B,C]
    out   = (tanh(tanh(codes@W4+b4)@W5+b5)) @ W6 + b6   # [B,T]

Key algebraic restructuring: codes = (sum_s mask*h2) @ W3 + N_b * b3, so the
third big matmul collapses to a [B,H]x[H,C] — only the two big MLP layers run
on all rows.  The device computes hsum[b] = sum_{s valid} h2[b,s,:]; the tiny
decode (<0.01% of FLOPs) runs on host.

Raggedness: valid rows are packed contiguously on host (segment sum is
permutation invariant) and split evenly over the 8 cores; a per-core selection
matrix sel[row, seg] (0/1) encodes both set membership and the validity mask,
applied as a matmul.  Cores run an identical program (SPMD) on different data.

Device layout per core (R rows, padded with zeros):
    a0  = words^T          [E on partitions, rows free]   (host pre-transposed)
    L1: psum[h,r] = sum_e W1[e,h] a0[e,r]; a1 = tanh(psum + b1)   (bias per
        partition on ScalarE)                                  -> [h, r]
    L2: psum[r,h] = sum_h' a1[h',r] W2[h',h]  (activation tile is the
        stationary operand so the output lands in natural [r,h] layout);
        += b2 broadcast on VectorE; a2 = tanh() on ScalarE     -> [r, h]
    seg: codes[s,h] += sel[r,s]^T a2[r,h] via matmul, accumulated in PSUM per
        row-tile and drained into an SBUF accumulator on VectorE.
All big matmuls use float32r (full PE rate at N=512, ~1e-4 relative error).
"""

import sys

if "/opt/trn_rl_repo" not in sys.path:
    sys.path.insert(0, "/opt/trn_rl_repo")

import ml_dtypes
import numpy as np

import concourse.bass as bass
import concourse.mybir as mybir
import concourse.tile as tile
from concourse import bacc
from concourse.bass_utils import run_bass_kernel_spmd

B, S, E = 64, 1024, 512
H = 512
NCORES = 8
P = 128
RT = 512  # rows per row-tile (matmul moving dim)
KC = E // P  # 4 contraction chunks

f32 = mybir.dt.float32
f32r = mybir.dt.float32r
bf16 = mybir.dt.bfloat16

_cache: dict = {}


def _tiles_of(R: int):
    """Row-tile sizes: full 512s plus an optional 256 remainder (fp32r needs
    the moving dim >=256 for full PE rate)."""
    assert R % 256 == 0
    return [RT] * (R // RT) + ([256] if R % RT else [])


def _build(R: int, SPAD: int):
    """Build + compile the SPMD program for R packed rows and SPAD segment
    columns per core."""
    key = (R, SPAD)
    if key in _cache:
        return _cache[key]

    tiles = _tiles_of(R)
    nt = len(tiles)
    nc = bacc.Bacc("TRN2", target_bir_lowering=False, debug=False, num_devices=NCORES)

    wT_d = nc.dram_tensor("wT", [P, KC, R], f32, kind="ExternalInput").ap()
    sel_d = nc.dram_tensor("sel", [R, SPAD], f32, kind="ExternalInput").ap()
    w1_d = nc.dram_tensor("w1", [E, H], f32, kind="ExternalInput").ap()
    w2_d = nc.dram_tensor("w2", [H, H], f32, kind="ExternalInput").ap()
    b1_d = nc.dram_tensor("b1", [H], f32, kind="ExternalInput").ap()
    b2b_d = nc.dram_tensor("b2b", [P, H], f32, kind="ExternalInput").ap()
    out_d = nc.dram_tensor("hsum", [nt, SPAD, H], f32, kind="ExternalOutput").ap()

    with tile.TileContext(nc) as tc:
        with (
            tc.tile_pool(name="const", bufs=1) as cpool,
            tc.tile_pool(name="a0", bufs=3) as a0pool,
            tc.tile_pool(name="a1", bufs=3) as a1pool,
            tc.tile_pool(name="a2", bufs=6) as a2pool,
            tc.tile_pool(name="ps1", bufs=3, space="PSUM") as ps1pool,
            tc.tile_pool(name="ps2", bufs=3, space="PSUM") as ps2pool,
            tc.tile_pool(name="ps3", bufs=2, space="PSUM") as ps3pool,
        ):
            # PE warmup: dependency-free bf16 matmuls issued first so the HAM
            # clock-gate opens (1.2 -> 2.4 GHz) while the first DMAs land.
            warm_sb = cpool.tile([P, RT], bf16)
            nc.gpsimd.memset(warm_sb[:], 0.25)
            for w in range(14):
                wps = ps1pool.tile([P, RT], f32, tag="ps1", name="wps")
                nc.tensor.matmul(wps[:], warm_sb[:, :P], warm_sb[:], start=True, stop=True)

            # DMA issue order = critical path first, at k-chunk granularity:
            # the first L1 matmul only needs w1[k=0] + a0[t=0][k=0].
            offs = [sum(tiles[:i]) for i in range(nt)]
            b1sb = cpool.tile([P, KC], f32)
            nc.sync.dma_start(b1sb[:], b1_d.rearrange("(hc p) -> p hc", p=P))
            w1k = []
            w2k = []
            a0_tiles = {}
            a0_tiles[0] = []
            nr0 = tiles[0]
            for k in range(KC):
                w1c = cpool.tile([P, H], f32r, name=f"w1k{k}")
                nc.sync.dma_start(w1c[:], w1_d[k * P:(k + 1) * P, :].bitcast(f32r))
                w1k.append(w1c)
                a0c = a0pool.tile([P, RT], f32r, tag=f"a0k{k}", name=f"a0k{k}")
                nc.sync.dma_start(
                    a0c[:, :nr0], wT_d[:, k, offs[0]:offs[0] + nr0].bitcast(f32r)
                )
                a0_tiles[0].append(a0c)
            for k in range(KC):
                w2c = cpool.tile([P, H], f32r, name=f"w2k{k}")
                nc.sync.dma_start(w2c[:], w2_d[k * P:(k + 1) * P, :].bitcast(f32r))
                w2k.append(w2c)
            if nt > 1:
                nr1 = tiles[1]
                a0_tiles[1] = []
                for k in range(KC):
                    a0c = a0pool.tile([P, RT], f32r, tag=f"a0k{k}", name=f"a0k{k}")
                    nc.sync.dma_start(
                        a0c[:, :nr1], wT_d[:, k, offs[1]:offs[1] + nr1].bitcast(f32r)
                    )
                    a0_tiles[1].append(a0c)
            # non-critical constants go through the gpsimd (SWDGE) queue so
            # they don't serialize behind the startup-critical sync-queue DMAs
            b2sb = cpool.tile([P, H], f32)
            nc.gpsimd.dma_start(b2sb[:], b2b_d)
            selsb = cpool.tile([P, R // P, SPAD], f32r)
            nc.gpsimd.dma_start(
                selsb[:], sel_d.rearrange("(rs p) s -> p rs s", p=P).bitcast(f32r)
            )

            for t in range(nt):
                nr = tiles[t]
                nsub = nr // P
                if t in a0_tiles:
                    a0 = a0_tiles.pop(t)

                    def a0k(k, a0=a0):
                        return a0[k]
                else:
                    a0m = a0pool.tile([P, KC, RT], f32r, tag="a0m", name="a0m")
                    nc.sync.dma_start(
                        a0m[:, :, :nr],
                        wT_d[:, :, offs[t]:offs[t] + nr].bitcast(f32r),
                    )

                    def a0k(k, a0m=a0m):
                        return a0m[:, k]
                # --- L1: transposed output [h, r] ---
                a1 = a1pool.tile([P, KC, RT], f32r, tag="a1")
                for m in range(KC):
                    ps = ps1pool.tile([P, RT], f32, tag="ps1")
                    for k in range(KC):
                        nc.tensor.matmul(
                            ps[:, :nr],
                            w1k[k][:, m * P:(m + 1) * P],
                            a0k(k)[:, :nr],
                            start=(k == 0),
                            stop=(k == KC - 1),
                        )
                    nc.scalar.activation(
                        a1[:, m, :nr],
                        ps[:, :nr],
                        mybir.ActivationFunctionType.Tanh,
                        bias=b1sb[:, m:m + 1],
                    )
                # --- L2: natural output [r, h], 128-row subtiles ---
                a2s = []
                for rs in range(nsub):
                    ps2 = ps2pool.tile([P, H], f32, tag="ps2")
                    for k in range(KC):
                        nc.tensor.matmul(
                            ps2[:],
                            a1[:, k, rs * P:(rs + 1) * P],
                            w2k[k][:],
                            start=(k == 0),
                            stop=(k == KC - 1),
                        )
                    nc.vector.tensor_add(ps2[:], ps2[:], b2sb[:])
                    a2 = a2pool.tile([P, H], f32r, tag="a2")
                    nc.scalar.activation(
                        a2[:], ps2[:], mybir.ActivationFunctionType.Tanh
                    )
                    a2s.append(a2)
                # --- segment sum partial: out[t] = sel^T @ a2 (host sums) ---
                ps3 = ps3pool.tile([SPAD, H], f32, tag="ps3")
                for rs in range(nsub):
                    nc.tensor.matmul(
                        ps3[:],
                        selsb[:, offs[t] // P + rs, :],
                        a2s[rs][:],
                        start=(rs == 0),
                        stop=(rs == nsub - 1),
                    )
                seg_out = a2pool.tile([SPAD, H], f32, tag="segout", name="seg_out")
                nc.vector.tensor_copy(seg_out[:], ps3[:])
                nc.sync.dma_start(out_d[t], seg_out[:])

    nc.compile()
    _cache[key] = nc
    return nc


def _pack(words: np.ndarray):
    """Pack valid rows contiguously, split across cores.

    Returns per-core arrays + bookkeeping to scatter partial segment sums back
    to global set ids.
    """
    words = np.asarray(words, dtype=np.float32)
    mask = np.sign(np.abs(words.sum(axis=-1)))  # [B, S], matches reference
    valid = mask > 0

    rows = []
    segs = []
    for b in range(B):
        vb = words[b][valid[b]]
        rows.append(vb)
        segs.append(np.full(len(vb), b, dtype=np.int64))
    rows = np.concatenate(rows, axis=0)
    segs = np.concatenate(segs, axis=0)
    total = len(rows)

    quota = -(-total // NCORES)  # ceil
    R = -(-quota // 256) * 256  # pad to tile granularity
    cores = []
    spad_needed = 1
    for c in range(NCORES):
        lo, hi = c * quota, min((c + 1) * quota, total)
        chunk = rows[lo:hi]
        seg_chunk = segs[lo:hi]
        n = hi - lo
        if n < R:
            chunk = np.concatenate(
                [chunk, np.zeros((R - n, E), dtype=np.float32)], axis=0
            )
        gids = []
        col_of = {}
        cols = np.zeros(n, dtype=np.int64)
        for i, g in enumerate(seg_chunk):
            if g not in col_of:
                col_of[g] = len(gids)
                gids.append(int(g))
            cols[i] = col_of[g]
        spad_needed = max(spad_needed, len(gids))
        cores.append((chunk, cols, n, gids))

    SPAD = max(8, -(-spad_needed // 8) * 8)
    assert SPAD <= P, f"too many segments per core: {spad_needed}"

    per_core = []
    for chunk, cols, n, gids in cores:
        wT = np.ascontiguousarray(
            chunk.T.reshape(KC, P, R).transpose(1, 0, 2)
        )  # [P, KC, R]
        sel = np.zeros((R, SPAD), dtype=np.float32)
        if n:
            sel[np.arange(n), cols] = 1.0
        per_core.append((wT, sel, gids))
    return per_core, R, SPAD, mask


def kernel(words, W1, b1, W2, b2, W3, b3, W4, b4, W5, b5, W6, b6):
    per_core, R, SPAD, mask = _pack(words)
    nc = _build(R, SPAD)

    W1 = np.asarray(W1, dtype=np.float32)
    W2 = np.asarray(W2, dtype=np.float32)
    b1 = np.asarray(b1, dtype=np.float32)
    b2 = np.asarray(b2, dtype=np.float32)
    b2b = np.broadcast_to(b2[None, :], (P, H)).copy()

    in_maps = []
    for wT, sel, _gids in per_core:
        in_maps.append(
            {"wT": wT, "sel": sel, "w1": W1, "w2": W2, "b1": b1, "b2b": b2b}
        )

    res = run_bass_kernel_spmd(nc, in_maps, core_ids=list(range(NCORES)))

    hsum = np.zeros((B, H), dtype=np.float32)
    for c in range(NCORES):
        out_c = res.results[c]["hsum"].sum(axis=0)
        for j, g in enumerate(per_core[c][2]):
            hsum[g] += out_c[j]

    # host decode (tiny)
    lengths = mask.sum(axis=1).astype(np.float32)[:, None]
    codes = hsum @ np.asarray(W3, np.float32) + lengths * np.asarray(b3, np.float32)
    h = np.tanh(codes @ np.asarray(W4, np.float32) + np.asarray(b4, np.float32))
    h = np.tanh(h @ np.asarray(W5, np.float32) + np.asarray(b5, np.float32))
    out = h @ np.asarray(W6, np.float32) + np.asarray(b6, np.float32)
    return out.astype(np.float32)


# revision 16
# speedup vs baseline: 1.0864x; 1.0864x over previous
"""DeepSet baseline kernel for Trainium2 (8 NeuronCores, data-parallel).

Model (reference):
    mask  = sign(|sum_e words|)                  # padding rows are all-zero
    h1    = tanh(words @ W1 + b1)                # [B,S,H]
    h2    = tanh(h1 @ W2 + b2)                   # [B,S,H]
    enc   = h2 @ W3 + b3                         # [B,S,C]
    codes = sum_s enc * mask                     # [B,C]
    out   = (tanh(tanh(codes@W4+b4)@W5+b5)) @ W6 + b6   # [B,T]

Key algebraic restructuring: codes = (sum_s mask*h2) @ W3 + N_b * b3, so the
third big matmul collapses to a [B,H]x[H,C] — only the two big MLP layers run
over all rows.  The device computes hsum[b] = sum_{s valid} h2[b,s,:]; the
tiny decode (<0.01% of FLOPs) runs on host.

Raggedness: valid rows are packed contiguously on host (segment sum is
permutation invariant) and split evenly over the 8 cores; a per-core selection
matrix sel[row, seg] (0/1) encodes both set membership and the validity mask,
applied as a matmul.  Cores run an identical program (SPMD) on different data.

Device pipeline per core (R rows, zero padded):
    a0  = words^T          [E on partitions, rows free]   (host pre-transposed)
    L1: psum[h,r] = sum_e W1[e,h] a0[e,r]; a1 = tanh(psum + b1)   (bias is
        per-partition on ScalarE)                                -> [h, r]
    L2: psum[r,h] = sum_h' a1[h',r] W2[h',h]  (activation tile is the
        stationary operand so the output lands in natural [r,h] layout);
        += b2 broadcast on VectorE; a2 = tanh() on ScalarE       -> [r, h]
    seg: out[t][s,h] = sel[r,s]^T a2[r,h] via matmul accumulated in PSUM per
        row-tile, copied out per tile; host sums the partials.
L2/segment matmuls use float32r (full PE rate at N>=256, ~1e-4 error); L1
optionally runs in bf16 (BF16_L1) to halve the input DMA and enable fast
weight loads.
"""

import sys

if "/opt/trn_rl_repo" not in sys.path:
    sys.path.insert(0, "/opt/trn_rl_repo")

import ml_dtypes
import numpy as np

import concourse.bass as bass
import concourse.mybir as mybir
import concourse.tile as tile
from concourse import bacc
from concourse.bass_utils import run_bass_kernel_spmd

B, S, E = 64, 1024, 512
H = 512
NCORES = 8
P = 128
RT = 512  # rows per row-tile (matmul moving dim)
KC = E // P  # 4 contraction chunks

BF16_L1 = True  # words/W1 in bf16 (L1 only); L2 + segment stay fp32r
N_WARMUP = 14  # dep-free matmuls to open the HAM clock gate during DMA wait

f32 = mybir.dt.float32
f32r = mybir.dt.float32r
bf16 = mybir.dt.bfloat16

_cache: dict = {}


def _tiles_of(R: int):
    """Row-tile sizes: full 512s plus an optional 256 remainder (fp32r needs
    the moving dim >=256 for full PE rate)."""
    assert R % 256 == 0
    return [RT] * (R // RT) + ([256] if R % RT else [])


def _build(R: int, SPAD: int):
    key = (R, SPAD)
    if key in _cache:
        return _cache[key]

    tiles = _tiles_of(R)
    nt = len(tiles)
    offs = [sum(tiles[:i]) for i in range(nt)]
    in_dt = bf16 if BF16_L1 else f32
    l1_dt = bf16 if BF16_L1 else f32r

    nc = bacc.Bacc("TRN2", target_bir_lowering=False, debug=False, num_devices=NCORES)

    wT_d = nc.dram_tensor("wT", [P, KC, R], in_dt, kind="ExternalInput").ap()
    sel_d = nc.dram_tensor("sel", [R, SPAD], f32, kind="ExternalInput").ap()
    w1_d = nc.dram_tensor("w1", [E, H], in_dt, kind="ExternalInput").ap()
    w2_d = nc.dram_tensor("w2", [H, H], f32, kind="ExternalInput").ap()
    b1_d = nc.dram_tensor("b1", [H], f32, kind="ExternalInput").ap()
    b2b_d = nc.dram_tensor("b2b", [P, H], f32, kind="ExternalInput").ap()
    out_d = nc.dram_tensor("hsum", [nt, SPAD, H], f32, kind="ExternalOutput").ap()

    def cast_l1(ap):
        return ap if BF16_L1 else ap.bitcast(f32r)

    with tile.TileContext(nc) as tc:
        with (
            tc.tile_pool(name="const", bufs=1) as cpool,
            tc.tile_pool(name="a0", bufs=3) as a0pool,
            tc.tile_pool(name="a1", bufs=3) as a1pool,
            tc.tile_pool(name="a2", bufs=6) as a2pool,
            tc.tile_pool(name="ps1", bufs=3, space="PSUM") as ps1pool,
            tc.tile_pool(name="ps2", bufs=3, space="PSUM") as ps2pool,
            tc.tile_pool(name="ps3", bufs=2, space="PSUM") as ps3pool,
        ):
            # PE warmup: dependency-free bf16 matmuls issued first so the HAM
            # clock-gate opens (1.2 -> 2.4 GHz) while the first DMAs land.
            warm_sb = cpool.tile([P, RT], bf16)
            nc.gpsimd.memset(warm_sb[:], 0.25)
            for w in range(N_WARMUP):
                wps = ps1pool.tile([P, RT], f32, tag="ps1", name="wps")
                nc.tensor.matmul(
                    wps[:], warm_sb[:, :P], warm_sb[:], start=True, stop=True
                )

            # DMA issue order = critical path first, at k-chunk granularity:
            # the first L1 matmul only needs w1[k=0] + a0[t=0][k=0].
            b1sb = cpool.tile([P, KC], f32)
            nc.sync.dma_start(b1sb[:], b1_d.rearrange("(hc p) -> p hc", p=P))
            w1k = []
            w2k = []
            a0_pre = {0: [], 1: []}
            for k in range(KC):
                w1c = cpool.tile([P, H], l1_dt, name=f"w1k{k}")
                nc.sync.dma_start(w1c[:], cast_l1(w1_d[k * P:(k + 1) * P, :]))
                w1k.append(w1c)
                a0c = a0pool.tile([P, RT], l1_dt, tag=f"a0k{k}", name=f"a0k{k}")
                nc.sync.dma_start(
                    a0c[:, :tiles[0]],
                    cast_l1(wT_d[:, k, offs[0]:offs[0] + tiles[0]]),
                )
                a0_pre[0].append(a0c)
            for k in range(KC):
                w2c = cpool.tile([P, H], f32r, name=f"w2k{k}")
                nc.sync.dma_start(w2c[:], w2_d[k * P:(k + 1) * P, :].bitcast(f32r))
                w2k.append(w2c)
                if nt > 1:
                    a0c = a0pool.tile([P, RT], l1_dt, tag=f"a0k{k}", name=f"a0k{k}")
                    nc.sync.dma_start(
                        a0c[:, :tiles[1]],
                        cast_l1(wT_d[:, k, offs[1]:offs[1] + tiles[1]]),
                    )
                    a0_pre[1].append(a0c)
            # non-critical constants go via the gpsimd (SWDGE) queue so they
            # don't serialize behind the startup-critical sync-queue DMAs
            b2sb = cpool.tile([P, H], f32)
            nc.gpsimd.dma_start(b2sb[:], b2b_d)
            selsb = cpool.tile([P, R // P, SPAD], f32r)
            nc.gpsimd.dma_start(
                selsb[:], sel_d.rearrange("(rs p) s -> p rs s", p=P).bitcast(f32r)
            )

            for t in range(nt):
                nr = tiles[t]
                nsub = nr // P
                if t in a0_pre and a0_pre[t]:
                    a0 = a0_pre.pop(t)
                else:
                    a0 = []
                    for k in range(KC):
                        a0c = a0pool.tile(
                            [P, RT], l1_dt, tag=f"a0k{k}", name=f"a0k{k}"
                        )
                        nc.sync.dma_start(
                            a0c[:, :nr],
                            cast_l1(wT_d[:, k, offs[t]:offs[t] + nr]),
                        )
                        a0.append(a0c)
                # --- L1: transposed output [h, r] ---
                a1 = a1pool.tile([P, KC, RT], f32r, tag="a1")
                for m in range(KC):
                    ps = ps1pool.tile([P, RT], f32, tag="ps1")
                    for k in range(KC):
                        nc.tensor.matmul(
                            ps[:, :nr],
                            w1k[k][:, m * P:(m + 1) * P],
                            a0[k][:, :nr],
                            start=(k == 0),
                            stop=(k == KC - 1),
                        )
                    nc.scalar.activation(
                        a1[:, m, :nr],
                        ps[:, :nr],
                        mybir.ActivationFunctionType.Tanh,
                        bias=b1sb[:, m:m + 1],
                    )
                # --- L2: natural output [r, h], 128-row subtiles ---
                a2s = []
                for rs in range(nsub):
                    ps2 = ps2pool.tile([P, H], f32, tag="ps2")
                    for k in range(KC):
                        nc.tensor.matmul(
                            ps2[:],
                            a1[:, k, rs * P:(rs + 1) * P],
                            w2k[k][:],
                            start=(k == 0),
                            stop=(k == KC - 1),
                        )
                    nc.vector.tensor_add(ps2[:], ps2[:], b2sb[:])
                    a2 = a2pool.tile([P, H], f32r, tag="a2")
                    nc.scalar.activation(
                        a2[:], ps2[:], mybir.ActivationFunctionType.Tanh
                    )
                    a2s.append(a2)
                # --- segment sum partial: out[t] = sel^T @ a2 (host sums) ---
                ps3 = ps3pool.tile([SPAD, H], f32, tag="ps3")
                for rs in range(nsub):
                    nc.tensor.matmul(
                        ps3[:],
                        selsb[:, offs[t] // P + rs, :],
                        a2s[rs][:],
                        start=(rs == 0),
                        stop=(rs == nsub - 1),
                    )
                seg_out = a2pool.tile([SPAD, H], f32, tag="segout", name="seg_out")
                nc.vector.tensor_copy(seg_out[:], ps3[:])
                nc.sync.dma_start(out_d[t], seg_out[:])

    nc.compile()
    _cache[key] = nc
    return nc


def _pack(words: np.ndarray):
    """Pack valid rows contiguously, split across cores.

    Returns per-core arrays + bookkeeping to scatter partial segment sums back
    to global set ids.
    """
    words = np.asarray(words, dtype=np.float32)
    mask = np.sign(np.abs(words.sum(axis=-1)))  # [B, S], matches reference
    valid = mask > 0

    rows = []
    segs = []
    for b in range(B):
        vb = words[b][valid[b]]
        rows.append(vb)
        segs.append(np.full(len(vb), b, dtype=np.int64))
    rows = np.concatenate(rows, axis=0)
    segs = np.concatenate(segs, axis=0)
    total = len(rows)

    quota = -(-total // NCORES)  # ceil
    R = -(-quota // 256) * 256  # pad to tile granularity
    cores = []
    spad_needed = 1
    for c in range(NCORES):
        lo, hi = c * quota, min((c + 1) * quota, total)
        chunk = rows[lo:hi]
        seg_chunk = segs[lo:hi]
        n = hi - lo
        if n < R:
            chunk = np.concatenate(
                [chunk, np.zeros((R - n, E), dtype=np.float32)], axis=0
            )
        gids = []
        col_of = {}
        cols = np.zeros(n, dtype=np.int64)
        for i, g in enumerate(seg_chunk):
            if g not in col_of:
                col_of[g] = len(gids)
                gids.append(int(g))
            cols[i] = col_of[g]
        spad_needed = max(spad_needed, len(gids))
        cores.append((chunk, cols, n, gids))

    SPAD = max(8, -(-spad_needed // 8) * 8)
    assert SPAD <= P, f"too many segments per core: {spad_needed}"

    in_np = ml_dtypes.bfloat16 if BF16_L1 else np.float32
    per_core = []
    for chunk, cols, n, gids in cores:
        wT = np.ascontiguousarray(
            chunk.T.reshape(KC, P, R).transpose(1, 0, 2)
        ).astype(in_np)  # [P, KC, R]
        sel = np.zeros((R, SPAD), dtype=np.float32)
        if n:
            sel[np.arange(n), cols] = 1.0
        per_core.append((wT, sel, gids))
    return per_core, R, SPAD, mask


def _in_maps(per_core, inputs):
    W1 = np.asarray(inputs["W1"], dtype=ml_dtypes.bfloat16 if BF16_L1 else np.float32)
    W2 = np.asarray(inputs["W2"], dtype=np.float32)
    b1 = np.asarray(inputs["b1"], dtype=np.float32)
    b2 = np.asarray(inputs["b2"], dtype=np.float32)
    b2b = np.broadcast_to(b2[None, :], (P, H)).copy()
    return [
        {"wT": wT, "sel": sel, "w1": W1, "w2": W2, "b1": b1, "b2b": b2b}
        for (wT, sel, _g) in per_core
    ]


def kernel(words, W1, b1, W2, b2, W3, b3, W4, b4, W5, b5, W6, b6):
    per_core, R, SPAD, mask = _pack(words)
    nc = _build(R, SPAD)
    in_maps = _in_maps(
        per_core, {"W1": W1, "W2": W2, "b1": b1, "b2": b2}
    )

    res = run_bass_kernel_spmd(nc, in_maps, core_ids=list(range(NCORES)))

    hsum = np.zeros((B, H), dtype=np.float32)
    for c in range(NCORES):
        out_c = res.results[c]["hsum"].sum(axis=0)
        for j, g in enumerate(per_core[c][2]):
            hsum[g] += out_c[j]

    # host decode (tiny)
    lengths = mask.sum(axis=1).astype(np.float32)[:, None]
    codes = hsum @ np.asarray(W3, np.float32) + lengths * np.asarray(b3, np.float32)
    h = np.tanh(codes @ np.asarray(W4, np.float32) + np.asarray(b4, np.float32))
    h = np.tanh(h @ np.asarray(W5, np.float32) + np.asarray(b5, np.float32))
    out = h @ np.asarray(W6, np.float32) + np.asarray(b6, np.float32)
    return out.astype(np.float32)
